# revision 40
# baseline (speedup 1.0000x reference)
"""Multi-head causal attention (B=4, S=2048, D=1024, H=16) on 8 TRN2 NeuronCores.

Sharding: core c -> (batch c//2, head-group c%2 of 8 heads = 512 d_model cols).
Each core:
  - projects Q/K/V for its head slice (bf16 matmuls, fp32 accum)
  - causal attention for its 8 heads over the full sequence, computed with
    scores transposed ([keys, q]) so exp(scores)^T feeds the A@V matmul as the
    moving operand.  Head pairs are processed two at a time ("dyads"): per key
    tile the two ctx passes are col-tiled (head-even -> PE cols 0-63, head-odd
    -> 64-127, concurrent), and ONE extra quad-sums pass (4 x M=32 col-tiled
    ones matmuls) accumulates all four heads' softmax sums
  - normalization per pair: sums -> SBUF, reciprocal, multiply out of PSUM
  - partial out-projection ctx^T @ Wo[rows-of-its-heads]  (no bias)
Host: out[b] = partial[2b] + partial[2b+1] + bo.

DMA loads are batched into a few large descriptors (host pre-packs weights)
since DMA triggers serialize on the Sync sequencer at ~600ns each; x chunks
are prefetched one chunk ahead.  Each chunk's own kT/V projections run as PE
filler inside the chunk (deadline-paced before the diagonal needs them), so
late chunks - where exp dominates - keep PE filler work available.
"""

import numpy as np
import ml_dtypes
from contextlib import ExitStack

import concourse.bass as bass
import concourse.tile as tile
from concourse import bacc, mybir
from concourse.bass_utils import run_bass_kernel_spmd

F32 = mybir.dt.float32
BF16 = mybir.dt.bfloat16
I32 = mybir.dt.int32
EXP = mybir.ActivationFunctionType.Exp

# Schraudolph exp bit-trick in bf16 bit-layout: exp(s*SCALE) ~=
#   bitcast_bf16(i16(s * SK16 + SB16)); ~2% RMS error.  One DVE
# tensor_scalar per tile offloads softmax exp from the saturated Scalar
# engine; the ctx matmul reads the int16 tile bitcast as bf16.
SK16 = 0.125 * 1.4426950408889634 * 128.0
SB16 = 127.0 * 128.0 - 366393.0 / 65536.0 + 0.5

N_CORES = 8
S = 2048          # sequence length
D = 1024          # d_model
HL = 8            # heads per core
HD = 64           # head dim
DL = HL * HD      # local d_model slice = 512
SCALE = 1.0 / 8.0  # 1/sqrt(HD)

NQC = S // 512    # 4 q chunks of 512
NDT = D // 128    # 8 d_model(in) tiles
NMT = DL // 128   # 4 local dout tiles (head pairs)

_compiled = None  # cached (nc,) so repeated kernel() calls skip rebuild


def _build():
    nc = bacc.Bacc("TRN2", target_bir_lowering=False, debug=False,
                   num_devices=N_CORES)

    # host-packed inputs (see _shard):
    #   wqkv: [3*D, DL] bf16 (Wq|Wk|Wv rows), wo: [DL, D] bf16
    #   bqk:  [128, 8] f32 (bq m-tiles in cols 0-3, bk in cols 4-7)
    #   bvb:  [128, DL] f32 (bv broadcast over partitions)
    #   xqt/xkt/xvt: [D, S] bf16 (x transposed)
    xq_ap = nc.dram_tensor("xqt", [D, S], BF16, kind="ExternalInput").ap()
    xk_ap = nc.dram_tensor("xkt", [D, S], BF16, kind="ExternalInput").ap()
    xv_ap = nc.dram_tensor("xvt", [D, S], BF16, kind="ExternalInput").ap()
    wqkv_ap = nc.dram_tensor("wqkv", [3 * D, DL], BF16, kind="ExternalInput").ap()
    bqk_ap = nc.dram_tensor("bqk", [128, 2 * NMT], F32, kind="ExternalInput").ap()
    bvb_ap = nc.dram_tensor("bvb", [128, DL], F32, kind="ExternalInput").ap()
    wo_ap = nc.dram_tensor("wo", [DL, D], BF16, kind="ExternalInput").ap()
    out_ap = nc.dram_tensor("out", [S, D], BF16, kind="ExternalOutput").ap()

    with tile.TileContext(nc) as tc, ExitStack() as ctx:
        wpool = ctx.enter_context(tc.tile_pool(name="weights", bufs=1))
        xt_pool = ctx.enter_context(tc.tile_pool(name="xt", bufs=7))
        qkv_pool = ctx.enter_context(tc.tile_pool(name="qkv", bufs=1))
        exp_pool = ctx.enter_context(tc.tile_pool(name="expt", bufs=6))
        sch_pool = ctx.enter_context(tc.tile_pool(name="sch", bufs=3))
        norm_pool = ctx.enter_context(tc.tile_pool(name="norm", bufs=2))
        outst_pool = ctx.enter_context(tc.tile_pool(name="outst", bufs=2))
        # PSUM: scores/proj pool 2 x [128,1024] (4 banks) + ctx accumulators
        # 2 x [128,512] (2 banks) + quad-sums accumulators 2 x (2 banks)
        psum_big = ctx.enter_context(tc.tile_pool(name="ps_big", bufs=2, space="PSUM"))
        psum_ctx = ctx.enter_context(tc.tile_pool(name="ps_ctx", bufs=2, space="PSUM"))
        psum_sum = ctx.enter_context(tc.tile_pool(name="ps_sum", bufs=2, space="PSUM"))

        # ---- batched weight / bias loads ----
        # wqkv -> [128, 3, 8, 512]: (p,(i,d,c)) <- dram row 1024*i+128*d+p.
        # Issued as 3 DMAs interleaved with the x chunk-0 loads (emitted just
        # below) so the first projection's inputs (wq + xq0) complete first.
        wqkv_sb = wpool.tile([128, 3, NDT, DL], BF16, tag="wqkv")
        wqkv_src = wqkv_ap.rearrange("(i d p) c -> p i d c", i=3, d=NDT)
        wq_sb = [wqkv_sb[:, 0, d, :] for d in range(NDT)]
        wk_sb = [wqkv_sb[:, 1, d, :] for d in range(NDT)]
        wv_sb = [wqkv_sb[:, 2, d, :] for d in range(NDT)]

        # ---- x^T chunk load: one DMA per (input, chunk) -> [128, 8, 512] ----
        def load_xt_chunk(x_ap, qc, nm):
            t = xt_pool.tile([128, NDT, 512], BF16, tag="xt", name=f"{nm}xt{qc}")
            nc.sync.dma_start(
                t[:],
                x_ap[:, 512 * qc:512 * (qc + 1)].rearrange(
                    "(d p) c -> p d c", d=NDT))
            return t

        # chunk-0 loads
        nc.sync.dma_start(wqkv_sb[:], wqkv_src)
        xq_c0 = load_xt_chunk(xq_ap, 0, "q")
        xk_c0 = load_xt_chunk(xk_ap, 0, "k")
        xv_c0 = load_xt_chunk(xv_ap, 0, "v")

        bqk_sb = wpool.tile([128, 2 * NMT], F32, tag="bqk")
        nc.sync.dma_start(bqk_sb[:], bqk_ap)
        bvb_sb = wpool.tile([128, DL], F32, tag="bvb")
        nc.sync.dma_start(bvb_sb[:], bvb_ap)

        # wo -> [128, 4, 1024]
        wo_sb4 = wpool.tile([128, NMT, D], BF16, tag="wo")
        nc.sync.dma_start(wo_sb4[:], wo_ap.rearrange("(d p) c -> p d c", d=NMT))
        wo_sb = [wo_sb4[:, d, :] for d in range(NMT)]

        # qT/kT: [DL, S] bf16 stored as NMT tiles [128, S]
        qT = [qkv_pool.tile([128, S], BF16, tag=f"qT{m}", name=f"qT{m}") for m in range(NMT)]
        kT = [qkv_pool.tile([128, S], BF16, tag=f"kT{m}", name=f"kT{m}") for m in range(NMT)]

        def proj_chunk(xt, w_sb, bcol, res, qc, m):
            ps = psum_big.tile([128, 1024], F32, tag="big", name="ps")
            for d in range(NDT):
                nc.tensor.matmul(
                    ps[:, 0:512], w_sb[d][:, 128 * m:128 * (m + 1)],
                    xt[:, d, :],
                    start=(d == 0), stop=(d == NDT - 1))
            nc.vector.tensor_scalar_add(
                res[m][:, 512 * qc:512 * (qc + 1)], ps[:, 0:512],
                bqk_sb[:, bcol:bcol + 1])

        # v: straight [128, 512] per seq tile (head h -> cols 64h..64h+63);
        # softmax sums come from a separate quad-sums matmul pass against a
        # shared ones block (M=32 per head, col-tiled 4 heads per pass)
        v_sb = [qkv_pool.tile([128, DL], BF16, tag=f"v{st}", name=f"v{st}")
                for st in range(S // 128)]
        ones_sb = wpool.tile([128, 32], BF16, tag="ones")
        nc.vector.memset(ones_sb[:], 1.0)

        def v_chunk(xt, qc, sti):
            st = 4 * qc + sti
            ps = psum_big.tile([128, 1024], F32, tag="big", name="ps")
            for d in range(NDT):
                nc.tensor.matmul(ps[:, 0:DL], xt[:, d, 128 * sti:128 * (sti + 1)],
                                 wv_sb[d][:], start=(d == 0), stop=(d == NDT - 1))
            nc.vector.tensor_add(v_sb[st][:], ps[:, 0:DL], bvb_sb[:])

        # ---- out projection ----
        ctxT = [qkv_pool.tile([128, S], BF16, tag=f"ctxT{m}", name=f"ctxT{m}") for m in range(NMT)]

        def emit_outproj(qt):
            # evacuation via ScalarE Copy (DVE is busier than ScalarE here)
            ot = outst_pool.tile([128, 1024], BF16, tag="ot", name="ot")
            for n in range(2):
                po_ps = psum_big.tile([128, 1024], F32, tag="big", name="po_ps")
                for d in range(NMT):
                    nc.tensor.matmul(
                        po_ps[:, 0:512],
                        ctxT[d][:, 128 * qt:128 * (qt + 1)],
                        wo_sb[d][:, 512 * n:512 * (n + 1)],
                        start=(d == 0), stop=(d == NMT - 1))
                nc.scalar.activation(ot[:, 512 * n:512 * (n + 1)], po_ps[:, 0:512],
                                     mybir.ActivationFunctionType.Copy)
            nc.sync.dma_start(out_ap[128 * qt:128 * (qt + 1), :], ot[:])

        # ---- chunk 0 projections upfront (diagonal starts at kt=0) ----
        for m in range(NMT):
            proj_chunk(xq_c0, wq_sb, m, qT, 0, m)
        for m in range(NMT):
            proj_chunk(xk_c0, wk_sb, NMT + m, kT, 0, m)
        for sti in range(4):
            v_chunk(xv_c0, 0, sti)

        # x prefetch state: chunk qc's x tiles are loaded during chunk qc-1
        xk_next = xv_next = None

        # ---- attention per q-chunk ----
        for qc in range(NQC):
            xk_cur, xv_cur = xk_next, xv_next
            nkt = 4 * (qc + 1)
            ndiag = 4 * qc
            G = 2 * nkt  # global iteration count for this chunk (2 dyads)

            # filler list: (deadline_g, closure); sorted by deadline.
            # Deadlines are emission-order iterations g = dy*nkt + kt by which
            # the closure must have been emitted (data-dependency order).
            items = []
            BIG = 10 ** 6
            if qc > 0:
                # this chunk's v tiles (st >= 1): ctx(kt=ndiag+s) emitted at
                # python iteration ndiag+s+2 (pend depth) of dyad 0
                for s in range(1, 4):
                    items.append((min(ndiag + s, nkt),
                                  lambda s=s, x=xv_cur: v_chunk(x, qc, s)))
                # this chunk's kT m=2,3 (dyad 1): scores(kt=ndiag) of pair m
                # emitted at g = nkt + ndiag; margin 2.  (m=0,1 are hoisted
                # into the previous chunk.)
                for m in range(2, NMT):
                    items.append((nkt + ndiag - 2,
                                  lambda m=m, x=xk_cur: proj_chunk(x, wk_sb, NMT + m, kT, qc, m)))
            if qc + 1 < NQC:
                xq_c = load_xt_chunk(xq_ap, qc + 1, "q")
                xk_next = load_xt_chunk(xk_ap, qc + 1, "k")
                xv_next = load_xt_chunk(xv_ap, qc + 1, "v")
                for m in range(NMT):
                    items.append((BIG, lambda m=m, x=xq_c, q=qc + 1: proj_chunk(x, wq_sb, m, qT, q, m)))
                # hoist next chunk's first two kT groups / first v tile
                items.append((BIG, lambda x=xk_next, q=qc + 1: proj_chunk(x, wk_sb, NMT + 0, kT, q, 0)))
                items.append((BIG, lambda x=xk_next, q=qc + 1: proj_chunk(x, wk_sb, NMT + 1, kT, q, 1)))
                items.append((BIG, lambda x=xv_next, q=qc + 1: v_chunk(x, q, 0)))
            if qc > 0:
                items += [(BIG, lambda qt=4 * (qc - 1) + j: emit_outproj(qt))
                          for j in range(4)]
            items.sort(key=lambda it: it[0])
            nit = len(items)
            ndone = 0

            # two dyads of two head-pairs; kt is the inner loop so the quad
            # softmax-sums pass (4 x M=32 col-tiled) covers 4 heads at once
            for dy in range(2):
                pairs = (2 * dy, 2 * dy + 1)
                ctx_ps = {p: psum_ctx.tile([128, 512], F32, tag="ctx",
                                           name=f"ctx{qc}_{p}") for p in pairs}
                sums_ps = psum_sum.tile([128, 512], F32, tag="qsums",
                                        name=f"qsums{qc}_{dy}")

                def emit_scores_exp(p, kt):
                    qs = max(0, 128 * kt - 512 * qc)
                    sc_ps = psum_big.tile([128, 1024], F32, tag="big", name="sc")
                    for i in range(2):
                        po = 64 * i
                        nc.tensor.matmul(
                            sc_ps[:, 512 * i + qs:512 * (i + 1)],
                            kT[p][po:po + HD, 128 * kt:128 * (kt + 1)],
                            qT[p][po:po + HD, 512 * qc + qs:512 * (qc + 1)],
                            start=True, stop=True)
                    # late chunks are Scalar(exp)-bound: offload a slice of
                    # non-diagonal tiles to DVE via the exp bit-trick
                    offload = (qc == 2 and kt % 4 == 3 and kt < ndiag) or \
                              (qc == 3 and kt % 3 == 2 and kt < ndiag)
                    if offload:
                        it = sch_pool.tile([128, 1024], mybir.dt.int16,
                                           tag="it", name="it")
                        nc.vector.tensor_scalar(
                            it[:], sc_ps[:], SK16, SB16,
                            mybir.AluOpType.mult, mybir.AluOpType.add)
                        return it[:].bitcast(BF16)
                    et = exp_pool.tile([128, 1024], BF16, tag="et", name="et")
                    nc.scalar.activation(et[:, qs:1024], sc_ps[:, qs:1024],
                                         EXP, scale=SCALE)
                    return et

                def emit_ctx(kt, ets):
                    qs = max(0, 128 * kt - 512 * qc)
                    diag = ndiag <= kt
                    for i, p in enumerate(pairs):
                        et = ets[i]
                        for j in range(2):
                            h = 2 * p + j
                            if diag:  # mask k>q in the diagonal 128x128 block
                                nc.gpsimd.affine_select(
                                    out=et[:, 512 * j + qs:512 * j + qs + 128],
                                    in_=et[:, 512 * j + qs:512 * j + qs + 128],
                                    compare_op=mybir.AluOpType.is_ge, fill=0.0,
                                    base=0, pattern=[[1, 128]], channel_multiplier=-1)
                            # col-tiled pair: head-even -> psum rows 0-63,
                            # head-odd -> rows 64-127 (concurrent in the PE)
                            nc.tensor.matmul(
                                ctx_ps[p][64 * j:64 * (j + 1), qs:512],
                                v_sb[kt][:, HD * h:HD * (h + 1)],
                                et[:, 512 * j + qs:512 * (j + 1)],
                                start=(kt == 0), stop=(kt == nkt - 1))
                    # quad sums: 4 x M=32 col-tiled ones matmuls, one pass
                    for i in range(2):
                        for j in range(2):
                            qj = 2 * i + j
                            nc.tensor.matmul(
                                sums_ps[32 * qj:32 * (qj + 1), qs:512],
                                ones_sb[:],
                                ets[i][:, 512 * j + qs:512 * (j + 1)],
                                start=(kt == 0), stop=(kt == nkt - 1),
                                tile_position=(0, 32 * qj))

                # software pipeline: scores/exp run ahead of ctx
                pend = []
                for kt in range(nkt):
                    pend.append((kt, (emit_scores_exp(pairs[0], kt),
                                      emit_scores_exp(pairs[1], kt))))
                    if len(pend) > 2:
                        emit_ctx(*pend.pop(0))
                    # fillers: emit when due (deadline) or by uniform pacing
                    g = dy * nkt + kt + 1
                    want = (nit * g) // G
                    while ndone < nit and (ndone < want or items[ndone][0] <= g):
                        items[ndone][1]()
                        ndone += 1
                # force items due within this dyad before draining ctx
                lim = (dy + 1) * nkt
                while ndone < nit and items[ndone][0] <= lim:
                    items[ndone][1]()
                    ndone += 1
                for pn in pend:
                    emit_ctx(*pn)

                # normalization per pair: both heads' sums (64 rows) -> SBUF
                # base 0 (custom recip needs base-0 operands), one reciprocal,
                # then four 32-row multiplies straight out of PSUM (recip rows
                # 0-31 serve the even head, 32-63 the odd head)
                for i, p in enumerate(pairs):
                    s64 = norm_pool.tile([64, 512], F32, tag="sums", name="s64")
                    nc.vector.tensor_copy(s64[:], sums_ps[64 * i:64 * (i + 1), :])
                    r64 = norm_pool.tile([64, 512], F32, tag="recip", name="r64")
                    nc.vector.reciprocal_approx_fast(r64[:], s64[:])
                    for j in range(2):
                        for half in range(2):
                            rows = 64 * j + 32 * half
                            nc.vector.tensor_mul(
                                ctxT[p][rows:rows + 32, 512 * qc:512 * (qc + 1)],
                                ctx_ps[p][rows:rows + 32, :],
                                r64[32 * j:32 * (j + 1), :])

            while ndone < nit:
                items[ndone][1]()
                ndone += 1

        for qt in range(4 * (NQC - 1), 4 * NQC):
            emit_outproj(qt)

    nc.compile()
    return nc


def _shard(inputs):
    in_maps = []
    for c in range(N_CORES):
        b, g = c // 2, c % 2
        sl = slice(512 * g, 512 * (g + 1))
        wqkv = np.concatenate([
            inputs["Wq"][:, sl], inputs["Wk"][:, sl], inputs["Wv"][:, sl]],
            axis=0).astype(ml_dtypes.bfloat16)
        bqk = np.empty((128, 8), np.float32)
        for m in range(4):
            bqk[:, m] = inputs["bq"][sl][128 * m:128 * (m + 1)]
            bqk[:, 4 + m] = inputs["bk"][sl][128 * m:128 * (m + 1)]
        in_maps.append({
            "xqt": np.ascontiguousarray(inputs["inputs_q"][b].T.astype(ml_dtypes.bfloat16)),
            "xkt": np.ascontiguousarray(inputs["inputs_k"][b].T.astype(ml_dtypes.bfloat16)),
            "xvt": np.ascontiguousarray(inputs["inputs_v"][b].T.astype(ml_dtypes.bfloat16)),
            "wqkv": np.ascontiguousarray(wqkv),
            "bqk": bqk,
            "bvb": np.ascontiguousarray(
                np.broadcast_to(inputs["bv"][sl], (128, 512))).astype(np.float32),
            "wo": np.ascontiguousarray(inputs["Wo"][sl, :].astype(ml_dtypes.bfloat16)),
        })
    return in_maps


def kernel(**inputs):
    global _compiled
    inputs = {k: np.asarray(v, dtype=np.float32) for k, v in inputs.items()}
    if _compiled is None:
        _compiled = _build()
    nc = _compiled
    in_maps = _shard(inputs)
    res = run_bass_kernel_spmd(nc, in_maps, list(range(N_CORES)),
                               trace=bool(int(__import__("os").environ.get("BASS_TRACE", "0"))))
    kernel.last_results = res
    B = 4
    out = np.empty((B, S, D), np.float32)
    for b in range(B):
        out[b] = (res.results[2 * b]["out"].astype(np.float32)
                  + res.results[2 * b + 1]["out"].astype(np.float32))
    out += inputs["bo"][None, None, :]
    return out


# revision 44
# speedup vs baseline: 1.1724x; 1.1724x over previous
"""Multi-head causal attention (B=4, S=2048, D=1024, H=16) on 8 TRN2 NeuronCores.

Sharding: core c -> (batch c//2, head-group c%2 of 8 heads = 512 d_model cols).
Each core:
  - projects Q/K/V for its head slice (bf16 matmuls, fp32 accum)
  - causal attention for its 8 heads over the full sequence, computed with
    scores transposed ([keys, q]) so exp(scores)^T feeds the A@V matmul as the
    moving operand; the stationary is [V(64) | ones(64)] per head so ctx lands
    on psum rows 0-63 and the softmax sums replicated on rows 64-127 in one
    accumulation group
  - normalization: sums -> SBUF, reciprocal, multiply straight out of PSUM
  - partial out-projection ctx^T @ Wo[rows-of-its-heads]  (no bias)
Host: out[b] = partial[2b] + partial[2b+1] + bo.

DMA loads are batched into a few large descriptors (host pre-packs weights)
since DMA triggers serialize on the Sync sequencer at ~600ns each; x chunks
are prefetched one chunk ahead.  Each chunk's own kT/V projections run as PE
filler inside the chunk (deadline-paced before the diagonal needs them), so
late chunks - where exp dominates - keep PE filler work available.
"""

import numpy as np
import ml_dtypes
from contextlib import ExitStack

import concourse.bass as bass
import concourse.tile as tile
from concourse import bacc, mybir
from concourse.bass_utils import run_bass_kernel_spmd

F32 = mybir.dt.float32
BF16 = mybir.dt.bfloat16
I32 = mybir.dt.int32
EXP = mybir.ActivationFunctionType.Exp

# Schraudolph exp bit-trick in bf16 bit-layout: exp(s*SCALE) ~=
#   bitcast_bf16(i16(s * SK16 + SB16)); ~2% RMS error.  One DVE
# tensor_scalar per tile offloads softmax exp from the saturated Scalar
# engine; the ctx matmul reads the int16 tile bitcast as bf16.
SK16 = 0.125 * 1.4426950408889634 * 128.0
SB16 = 127.0 * 128.0 - 366393.0 / 65536.0 + 0.5

N_CORES = 8
S = 2048          # sequence length
D = 1024          # d_model
HL = 8            # heads per core
HD = 64           # head dim
DL = HL * HD      # local d_model slice = 512
SCALE = 1.0 / 8.0  # 1/sqrt(HD)

NQC = S // 512    # 4 q chunks of 512
NDT = D // 128    # 8 d_model(in) tiles
NMT = DL // 128   # 4 local dout tiles (head pairs)

_compiled = None  # cached (nc,) so repeated kernel() calls skip rebuild


def _build():
    nc = bacc.Bacc("TRN2", target_bir_lowering=False, debug=False,
                   num_devices=N_CORES)

    # host-packed inputs (see _shard):
    #   wqkv: [3*D, DL] bf16 (Wq|Wk|Wv rows), wo: [DL, D] bf16
    #   bqk:  [128, 8] f32 (bq m-tiles in cols 0-3, bk in cols 4-7)
    #   bvb:  [128, DL] f32 (bv broadcast over partitions)
    #   xqt/xkt/xvt: [D, S] bf16 (x transposed)
    xq_ap = nc.dram_tensor("xqt", [D, S], BF16, kind="ExternalInput").ap()
    xk_ap = nc.dram_tensor("xkt", [D, S], BF16, kind="ExternalInput").ap()
    xv_ap = nc.dram_tensor("xvt", [D, S], BF16, kind="ExternalInput").ap()
    wqkv_ap = nc.dram_tensor("wqkv", [3 * D, DL], BF16, kind="ExternalInput").ap()
    bqk_ap = nc.dram_tensor("bqk", [128, 2 * NMT], F32, kind="ExternalInput").ap()
    bvb_ap = nc.dram_tensor("bvb", [128, DL], F32, kind="ExternalInput").ap()
    wo_ap = nc.dram_tensor("wo", [DL, D], BF16, kind="ExternalInput").ap()
    out_ap = nc.dram_tensor("out", [S, D], BF16, kind="ExternalOutput").ap()

    with tile.TileContext(nc) as tc, ExitStack() as ctx:
        wpool = ctx.enter_context(tc.tile_pool(name="weights", bufs=1))
        xt_pool = ctx.enter_context(tc.tile_pool(name="xt", bufs=7))
        qkv_pool = ctx.enter_context(tc.tile_pool(name="qkv", bufs=1))
        exp_pool = ctx.enter_context(tc.tile_pool(name="expt", bufs=6))
        sch_pool = ctx.enter_context(tc.tile_pool(name="sch", bufs=3))
        norm_pool = ctx.enter_context(tc.tile_pool(name="norm", bufs=2))
        outst_pool = ctx.enter_context(tc.tile_pool(name="outst", bufs=2))
        # PSUM: scores/proj pool 3 x [128,1024] (6 banks) + ctx accumulators
        # 2 x [128,512] (2 banks)
        psum_big = ctx.enter_context(tc.tile_pool(name="ps_big", bufs=3, space="PSUM"))
        psum_ctx = ctx.enter_context(tc.tile_pool(name="ps_ctx", bufs=2, space="PSUM"))

        # ---- batched weight / bias loads ----
        # wqkv -> [128, 3, 8, 512]: (p,(i,d,c)) <- dram row 1024*i+128*d+p.
        # Issued as 3 DMAs interleaved with the x chunk-0 loads (emitted just
        # below) so the first projection's inputs (wq + xq0) complete first.
        wqkv_sb = wpool.tile([128, 3, NDT, DL], BF16, tag="wqkv")
        wqkv_src = wqkv_ap.rearrange("(i d p) c -> p i d c", i=3, d=NDT)
        wq_sb = [wqkv_sb[:, 0, d, :] for d in range(NDT)]
        wk_sb = [wqkv_sb[:, 1, d, :] for d in range(NDT)]
        wv_sb = [wqkv_sb[:, 2, d, :] for d in range(NDT)]

        # ---- x^T chunk load: one DMA per (input, chunk) -> [128, 8, 512] ----
        def load_xt_chunk(x_ap, qc, nm):
            t = xt_pool.tile([128, NDT, 512], BF16, tag="xt", name=f"{nm}xt{qc}")
            nc.sync.dma_start(
                t[:],
                x_ap[:, 512 * qc:512 * (qc + 1)].rearrange(
                    "(d p) c -> p d c", d=NDT))
            return t

        # chunk-0 loads
        nc.sync.dma_start(wqkv_sb[:], wqkv_src)
        xq_c0 = load_xt_chunk(xq_ap, 0, "q")
        xk_c0 = load_xt_chunk(xk_ap, 0, "k")
        xv_c0 = load_xt_chunk(xv_ap, 0, "v")

        bqk_sb = wpool.tile([128, 2 * NMT], F32, tag="bqk")
        nc.sync.dma_start(bqk_sb[:], bqk_ap)
        bvb_sb = wpool.tile([128, DL], F32, tag="bvb")
        nc.sync.dma_start(bvb_sb[:], bvb_ap)

        # wo -> [128, 4, 1024]
        wo_sb4 = wpool.tile([128, NMT, D], BF16, tag="wo")
        nc.sync.dma_start(wo_sb4[:], wo_ap.rearrange("(d p) c -> p d c", d=NMT))
        wo_sb = [wo_sb4[:, d, :] for d in range(NMT)]

        # qT/kT: [DL, S] bf16 stored as NMT tiles [128, S]
        qT = [qkv_pool.tile([128, S], BF16, tag=f"qT{m}", name=f"qT{m}") for m in range(NMT)]
        kT = [qkv_pool.tile([128, S], BF16, tag=f"kT{m}", name=f"kT{m}") for m in range(NMT)]

        def proj_chunk(xt, w_sb, bcol, res, qc, m):
            ps = psum_big.tile([128, 1024], F32, tag="big", name="ps")
            for d in range(NDT):
                nc.tensor.matmul(
                    ps[:, 0:512], w_sb[d][:, 128 * m:128 * (m + 1)],
                    xt[:, d, :],
                    start=(d == 0), stop=(d == NDT - 1))
            nc.vector.tensor_scalar_add(
                res[m][:, 512 * qc:512 * (qc + 1)], ps[:, 0:512],
                bqk_sb[:, bcol:bcol + 1])

        # v_aug: per seq-tile [128, HL, 2*HD] bf16; per head [V(64) | ones(64)]
        # so the ctx matmul (M=128, one accumulation group) leaves ctx on psum
        # rows 0-63 and the softmax sums replicated on rows 64-127.  The ones
        # halves are memset once upfront (DVE is idle during the initial DMA).
        v_aug = [qkv_pool.tile([128, HL, 2 * HD], BF16, tag=f"va{st}",
                               name=f"va{st}") for st in range(S // 128)]
        for st in range(S // 128):
            nc.vector.memset(v_aug[st][:, :, HD:2 * HD], 1.0)

        def v_chunk(xt, qc, sti):
            st = 4 * qc + sti
            ps = psum_big.tile([128, 1024], F32, tag="big", name="ps")
            for d in range(NDT):
                nc.tensor.matmul(ps[:, 0:DL], xt[:, d, 128 * sti:128 * (sti + 1)],
                                 wv_sb[d][:], start=(d == 0), stop=(d == NDT - 1))
            nc.vector.tensor_add(
                v_aug[st][:, :, 0:HD],
                ps[:, 0:DL].rearrange("p (h c) -> p h c", h=HL),
                bvb_sb[:].rearrange("p (h c) -> p h c", h=HL))

        # ---- out projection ----
        ctxT = [qkv_pool.tile([128, S], BF16, tag=f"ctxT{m}", name=f"ctxT{m}") for m in range(NMT)]

        def emit_outproj(qt):
            # evacuation via ScalarE Copy (DVE is busier than ScalarE here)
            ot = outst_pool.tile([128, 1024], BF16, tag="ot", name="ot")
            for n in range(2):
                po_ps = psum_big.tile([128, 1024], F32, tag="big", name="po_ps")
                for d in range(NMT):
                    nc.tensor.matmul(
                        po_ps[:, 0:512],
                        ctxT[d][:, 128 * qt:128 * (qt + 1)],
                        wo_sb[d][:, 512 * n:512 * (n + 1)],
                        start=(d == 0), stop=(d == NMT - 1))
                nc.scalar.activation(ot[:, 512 * n:512 * (n + 1)], po_ps[:, 0:512],
                                     mybir.ActivationFunctionType.Copy)
            nc.sync.dma_start(out_ap[128 * qt:128 * (qt + 1), :], ot[:])

        # ---- chunk 0 projections upfront (diagonal starts at kt=0) ----
        for m in range(NMT):
            proj_chunk(xq_c0, wq_sb, m, qT, 0, m)
        for m in range(NMT):
            proj_chunk(xk_c0, wk_sb, NMT + m, kT, 0, m)
        for sti in range(4):
            v_chunk(xv_c0, 0, sti)

        # x prefetch state: chunk qc's x tiles are loaded during chunk qc-1
        xk_next = xv_next = None

        # ---- attention per q-chunk ----
        for qc in range(NQC):
            xk_cur, xv_cur = xk_next, xv_next
            nkt = 4 * (qc + 1)
            ndiag = 4 * qc
            G = (HL // 2) * nkt  # global iteration count for this chunk

            # filler list: (deadline_g, closure); sorted by deadline.
            # Deadlines are emission-order iterations g = hp*nkt + kt by which
            # the closure must have been emitted (data-dependency order).
            items = []
            BIG = 10 ** 6
            if qc > 0:
                # this chunk's v tiles (st >= 1): ctx(kt=ndiag+s) emitted at
                # python iteration ndiag+s+4 (pend depth); keep margin 2
                for s in range(1, 4):
                    items.append((min(ndiag + s + 2, nkt),
                                  lambda s=s, x=xv_cur: v_chunk(x, qc, s)))
                # this chunk's kT m-groups (m >= 1): scores(kt=ndiag) of pair
                # m emitted at g = m*nkt + ndiag; margin 2
                for m in range(1, NMT):
                    items.append((m * nkt + ndiag - 2,
                                  lambda m=m, x=xk_cur: proj_chunk(x, wk_sb, NMT + m, kT, qc, m)))
            if qc + 1 < NQC:
                xq_c = load_xt_chunk(xq_ap, qc + 1, "q")
                xk_next = load_xt_chunk(xk_ap, qc + 1, "k")
                xv_next = load_xt_chunk(xv_ap, qc + 1, "v")
                for m in range(NMT):
                    items.append((BIG, lambda m=m, x=xq_c, q=qc + 1: proj_chunk(x, wq_sb, m, qT, q, m)))
                # hoist next chunk's first kT / v group into this chunk
                items.append((BIG, lambda x=xk_next, q=qc + 1: proj_chunk(x, wk_sb, NMT + 0, kT, q, 0)))
                items.append((BIG, lambda x=xv_next, q=qc + 1: v_chunk(x, q, 0)))
            if qc > 0:
                items += [(BIG, lambda qt=4 * (qc - 1) + j: emit_outproj(qt))
                          for j in range(4)]
            items.sort(key=lambda it: it[0])
            nit = len(items)
            ndone = 0

            for hp in range(HL // 2):
                heads = (2 * hp, 2 * hp + 1)
                ctx_ps = {h: psum_ctx.tile([128, 512], F32, tag="ctx",
                                           name=f"ctx{h}") for h in heads}

                def emit_scores_exp(kt):
                    qs = max(0, 128 * kt - 512 * qc)
                    sc_ps = psum_big.tile([128, 1024], F32, tag="big", name="sc")
                    for i in range(2):
                        po = 64 * i
                        nc.tensor.matmul(
                            sc_ps[:, 512 * i + qs:512 * (i + 1)],
                            kT[hp][po:po + HD, 128 * kt:128 * (kt + 1)],
                            qT[hp][po:po + HD, 512 * qc + qs:512 * (qc + 1)],
                            start=True, stop=True)
                    # late chunks are Scalar(exp)-bound: offload a slice of
                    # non-diagonal tiles to DVE via the exp bit-trick
                    offload = (qc == 2 and kt % 4 == 3 and kt < ndiag) or \
                              (qc == 3 and kt % 3 == 2 and kt < ndiag)
                    if offload:
                        it = sch_pool.tile([128, 1024], mybir.dt.int16,
                                           tag="it", name="it")
                        nc.vector.tensor_scalar(
                            it[:], sc_ps[:], SK16, SB16,
                            mybir.AluOpType.mult, mybir.AluOpType.add)
                        return it[:].bitcast(BF16)
                    et = exp_pool.tile([128, 1024], BF16, tag="et", name="et")
                    nc.scalar.activation(et[:, qs:1024], sc_ps[:, qs:1024],
                                         EXP, scale=SCALE)
                    return et

                def emit_ctx(kt, et):
                    qs = max(0, 128 * kt - 512 * qc)
                    diag = ndiag <= kt
                    for i, h in enumerate(heads):
                        if diag:  # mask k>q in the diagonal 128x128 block
                            nc.gpsimd.affine_select(
                                out=et[:, 512 * i + qs:512 * i + qs + 128],
                                in_=et[:, 512 * i + qs:512 * i + qs + 128],
                                compare_op=mybir.AluOpType.is_ge, fill=0.0,
                                base=0, pattern=[[1, 128]], channel_multiplier=-1)
                        nc.tensor.matmul(
                            ctx_ps[h][:, qs:512],
                            v_aug[kt][:, h, :],
                            et[:, 512 * i + qs:512 * (i + 1)],
                            start=(kt == 0), stop=(kt == nkt - 1))

                # software pipeline: scores/exp run ahead of ctx
                pend = []
                for kt in range(nkt):
                    pend.append((kt, emit_scores_exp(kt)))
                    if len(pend) > 4:
                        emit_ctx(*pend.pop(0))
                    # fillers: emit when due (deadline) or by uniform pacing
                    g = hp * nkt + kt + 1
                    want = (nit * g) // G
                    while ndone < nit and (ndone < want or items[ndone][0] <= g):
                        items[ndone][1]()
                        ndone += 1
                # force items due within this pair before draining ctx
                lim = (hp + 1) * nkt
                while ndone < nit and items[ndone][0] <= lim:
                    items[ndone][1]()
                    ndone += 1
                for pn in pend:
                    emit_ctx(*pn)

                # normalization: replicated sums (psum rows 64-127) -> SBUF
                # base 0 (the custom recip op requires base-0 operands),
                # reciprocal, then multiply straight out of PSUM
                for h in heads:
                    po = 64 * (h % 2)
                    sums = norm_pool.tile([HD, 512], F32, tag="sums", name="sums")
                    nc.vector.tensor_copy(sums[:], ctx_ps[h][64:128, :])
                    recip = norm_pool.tile([HD, 512], F32, tag="recip", name="recip")
                    nc.vector.reciprocal_approx_fast(recip[:], sums[:])
                    nc.vector.tensor_mul(
                        ctxT[hp][po:po + HD, 512 * qc:512 * (qc + 1)],
                        ctx_ps[h][0:HD, :], recip[:])

            while ndone < nit:
                items[ndone][1]()
                ndone += 1

        for qt in range(4 * (NQC - 1), 4 * NQC):
            emit_outproj(qt)

    nc.compile()
    return nc


def _shard(inputs):
    in_maps = []
    for c in range(N_CORES):
        b, g = c // 2, c % 2
        sl = slice(512 * g, 512 * (g + 1))
        wqkv = np.concatenate([
            inputs["Wq"][:, sl], inputs["Wk"][:, sl], inputs["Wv"][:, sl]],
            axis=0).astype(ml_dtypes.bfloat16)
        bqk = np.empty((128, 8), np.float32)
        for m in range(4):
            bqk[:, m] = inputs["bq"][sl][128 * m:128 * (m + 1)]
            bqk[:, 4 + m] = inputs["bk"][sl][128 * m:128 * (m + 1)]
        in_maps.append({
            "xqt": np.ascontiguousarray(inputs["inputs_q"][b].T.astype(ml_dtypes.bfloat16)),
            "xkt": np.ascontiguousarray(inputs["inputs_k"][b].T.astype(ml_dtypes.bfloat16)),
            "xvt": np.ascontiguousarray(inputs["inputs_v"][b].T.astype(ml_dtypes.bfloat16)),
            "wqkv": np.ascontiguousarray(wqkv),
            "bqk": bqk,
            "bvb": np.ascontiguousarray(
                np.broadcast_to(inputs["bv"][sl], (128, 512))).astype(np.float32),
            "wo": np.ascontiguousarray(inputs["Wo"][sl, :].astype(ml_dtypes.bfloat16)),
        })
    return in_maps


def kernel(**inputs):
    global _compiled
    inputs = {k: np.asarray(v, dtype=np.float32) for k, v in inputs.items()}
    if _compiled is None:
        _compiled = _build()
    nc = _compiled
    in_maps = _shard(inputs)
    res = run_bass_kernel_spmd(nc, in_maps, list(range(N_CORES)),
                               trace=bool(int(__import__("os").environ.get("BASS_TRACE", "0"))))
    kernel.last_results = res
    B = 4
    out = np.empty((B, S, D), np.float32)
    for b in range(B):
        out[b] = (res.results[2 * b]["out"].astype(np.float32)
                  + res.results[2 * b + 1]["out"].astype(np.float32))
    out += inputs["bo"][None, None, :]
    return out


# revision 45
# speedup vs baseline: 1.2294x; 1.0486x over previous
"""Multi-head causal attention (B=4, S=2048, D=1024, H=16) on 8 TRN2 NeuronCores.

Sharding: core c -> (batch c//2, head-group c%2 of 8 heads = 512 d_model cols).
Each core:
  - projects Q/K/V for its head slice (bf16 matmuls, fp32 accum)
  - causal attention for its 8 heads over the full sequence, computed with
    scores transposed ([keys, q]) so exp(scores)^T feeds the A@V matmul as the
    moving operand; the stationary is [V(64) | ones(64)] per head so ctx lands
    on psum rows 0-63 and the softmax sums replicated on rows 64-127 in one
    accumulation group
  - normalization: sums -> SBUF, reciprocal, multiply straight out of PSUM
  - partial out-projection ctx^T @ Wo[rows-of-its-heads]  (no bias)
Host: out[b] = partial[2b] + partial[2b+1] + bo.

DMA loads are batched into a few large descriptors (host pre-packs weights)
since DMA triggers serialize on the Sync sequencer at ~600ns each; x chunks
are prefetched one chunk ahead.  Each chunk's own kT/V projections run as PE
filler inside the chunk (deadline-paced before the diagonal needs them), so
late chunks - where exp dominates - keep PE filler work available.
"""

import numpy as np
import ml_dtypes
from contextlib import ExitStack

import concourse.bass as bass
import concourse.tile as tile
from concourse import bacc, mybir
from concourse.bass_utils import run_bass_kernel_spmd

F32 = mybir.dt.float32
BF16 = mybir.dt.bfloat16
I32 = mybir.dt.int32
EXP = mybir.ActivationFunctionType.Exp

# Schraudolph exp bit-trick in bf16 bit-layout: exp(s*SCALE) ~=
#   bitcast_bf16(i16(s * SK16 + SB16)); ~2% RMS error.  One DVE
# tensor_scalar per tile offloads softmax exp from the saturated Scalar
# engine; the ctx matmul reads the int16 tile bitcast as bf16.
SK16 = 0.125 * 1.4426950408889634 * 128.0
SB16 = 127.0 * 128.0 - 366393.0 / 65536.0 + 0.5

N_CORES = 8
S = 2048          # sequence length
D = 1024          # d_model
HL = 8            # heads per core
HD = 64           # head dim
DL = HL * HD      # local d_model slice = 512
SCALE = 1.0 / 8.0  # 1/sqrt(HD)

NQC = S // 512    # 4 q chunks of 512
NDT = D // 128    # 8 d_model(in) tiles
NMT = DL // 128   # 4 local dout tiles (head pairs)

_compiled = None  # cached (nc,) so repeated kernel() calls skip rebuild


def _build():
    nc = bacc.Bacc("TRN2", target_bir_lowering=False, debug=False,
                   num_devices=N_CORES)

    # host-packed inputs (see _shard):
    #   wqkv: [3*D, DL] bf16 (Wq|Wk|Wv rows), wo: [DL, D] bf16
    #   bqk:  [128, 8] f32 (bq m-tiles in cols 0-3, bk in cols 4-7)
    #   bvb:  [128, DL] f32 (bv broadcast over partitions)
    #   xqt/xkt/xvt: [D, S] bf16 (x transposed)
    xq_ap = nc.dram_tensor("xqt", [D, S], BF16, kind="ExternalInput").ap()
    xk_ap = nc.dram_tensor("xkt", [D, S], BF16, kind="ExternalInput").ap()
    xv_ap = nc.dram_tensor("xvt", [D, S], BF16, kind="ExternalInput").ap()
    wqkv_ap = nc.dram_tensor("wqkv", [3 * D, DL], BF16, kind="ExternalInput").ap()
    bqk_ap = nc.dram_tensor("bqk", [128, 2 * NMT], F32, kind="ExternalInput").ap()
    bvb_ap = nc.dram_tensor("bvb", [128, DL], F32, kind="ExternalInput").ap()
    wo_ap = nc.dram_tensor("wo", [DL, D], BF16, kind="ExternalInput").ap()
    out_ap = nc.dram_tensor("out", [S, D], BF16, kind="ExternalOutput").ap()

    with tile.TileContext(nc) as tc, ExitStack() as ctx:
        wpool = ctx.enter_context(tc.tile_pool(name="weights", bufs=1))
        xt_pool = ctx.enter_context(tc.tile_pool(name="xt", bufs=7))
        qkv_pool = ctx.enter_context(tc.tile_pool(name="qkv", bufs=1))
        exp_pool = ctx.enter_context(tc.tile_pool(name="expt", bufs=6))
        sch_pool = ctx.enter_context(tc.tile_pool(name="sch", bufs=3))
        norm_pool = ctx.enter_context(tc.tile_pool(name="norm", bufs=2))
        outst_pool = ctx.enter_context(tc.tile_pool(name="outst", bufs=2))
        # PSUM: scores pool 2 x [128,1024] (4 banks) + proj/outproj pool
        # 2 x [128,512] (2 banks, so filler bias-adds never block the scores
        # ring) + ctx accumulators 2 x [128,512] (2 banks)
        psum_big = ctx.enter_context(tc.tile_pool(name="ps_big", bufs=2, space="PSUM"))
        psum_proj = ctx.enter_context(tc.tile_pool(name="ps_proj", bufs=2, space="PSUM"))
        psum_ctx = ctx.enter_context(tc.tile_pool(name="ps_ctx", bufs=2, space="PSUM"))

        # ---- batched weight / bias loads ----
        # wqkv -> [128, 3, 8, 512]: (p,(i,d,c)) <- dram row 1024*i+128*d+p.
        # Issued as 3 DMAs interleaved with the x chunk-0 loads (emitted just
        # below) so the first projection's inputs (wq + xq0) complete first.
        wqkv_sb = wpool.tile([128, 3, NDT, DL], BF16, tag="wqkv")
        wqkv_src = wqkv_ap.rearrange("(i d p) c -> p i d c", i=3, d=NDT)
        wq_sb = [wqkv_sb[:, 0, d, :] for d in range(NDT)]
        wk_sb = [wqkv_sb[:, 1, d, :] for d in range(NDT)]
        wv_sb = [wqkv_sb[:, 2, d, :] for d in range(NDT)]

        # ---- x^T chunk load: one DMA per (input, chunk) -> [128, 8, 512] ----
        def load_xt_chunk(x_ap, qc, nm):
            t = xt_pool.tile([128, NDT, 512], BF16, tag="xt", name=f"{nm}xt{qc}")
            nc.sync.dma_start(
                t[:],
                x_ap[:, 512 * qc:512 * (qc + 1)].rearrange(
                    "(d p) c -> p d c", d=NDT))
            return t

        # chunk-0 loads
        nc.sync.dma_start(wqkv_sb[:], wqkv_src)
        xq_c0 = load_xt_chunk(xq_ap, 0, "q")
        xk_c0 = load_xt_chunk(xk_ap, 0, "k")
        xv_c0 = load_xt_chunk(xv_ap, 0, "v")

        bqk_sb = wpool.tile([128, 2 * NMT], F32, tag="bqk")
        nc.sync.dma_start(bqk_sb[:], bqk_ap)
        bvb_sb = wpool.tile([128, DL], F32, tag="bvb")
        nc.sync.dma_start(bvb_sb[:], bvb_ap)

        # wo -> [128, 4, 1024]
        wo_sb4 = wpool.tile([128, NMT, D], BF16, tag="wo")
        nc.sync.dma_start(wo_sb4[:], wo_ap.rearrange("(d p) c -> p d c", d=NMT))
        wo_sb = [wo_sb4[:, d, :] for d in range(NMT)]

        # qT/kT: [DL, S] bf16 stored as NMT tiles [128, S]
        qT = [qkv_pool.tile([128, S], BF16, tag=f"qT{m}", name=f"qT{m}") for m in range(NMT)]
        kT = [qkv_pool.tile([128, S], BF16, tag=f"kT{m}", name=f"kT{m}") for m in range(NMT)]

        def proj_chunk(xt, w_sb, bcol, res, qc, m):
            ps = psum_proj.tile([128, 512], F32, tag="proj", name="ps")
            for d in range(NDT):
                nc.tensor.matmul(
                    ps[:], w_sb[d][:, 128 * m:128 * (m + 1)],
                    xt[:, d, :],
                    start=(d == 0), stop=(d == NDT - 1))
            nc.vector.tensor_scalar_add(
                res[m][:, 512 * qc:512 * (qc + 1)], ps[:],
                bqk_sb[:, bcol:bcol + 1])

        # v_aug: per seq-tile [128, HL, 2*HD] bf16; per head [V(64) | ones(64)]
        # so the ctx matmul (M=128, one accumulation group) leaves ctx on psum
        # rows 0-63 and the softmax sums replicated on rows 64-127.  The ones
        # halves are memset once upfront (DVE is idle during the initial DMA).
        v_aug = [qkv_pool.tile([128, HL, 2 * HD], BF16, tag=f"va{st}",
                               name=f"va{st}") for st in range(S // 128)]
        for st in range(S // 128):
            nc.vector.memset(v_aug[st][:, :, HD:2 * HD], 1.0)

        def v_chunk(xt, qc, sti):
            st = 4 * qc + sti
            ps = psum_proj.tile([128, 512], F32, tag="proj", name="ps")
            for d in range(NDT):
                nc.tensor.matmul(ps[:], xt[:, d, 128 * sti:128 * (sti + 1)],
                                 wv_sb[d][:], start=(d == 0), stop=(d == NDT - 1))
            nc.vector.tensor_add(
                v_aug[st][:, :, 0:HD],
                ps[:].rearrange("p (h c) -> p h c", h=HL),
                bvb_sb[:].rearrange("p (h c) -> p h c", h=HL))

        # ---- out projection ----
        ctxT = [qkv_pool.tile([128, S], BF16, tag=f"ctxT{m}", name=f"ctxT{m}") for m in range(NMT)]

        def emit_outproj(qt):
            # evacuation via ScalarE Copy (DVE is busier than ScalarE here)
            ot = outst_pool.tile([128, 1024], BF16, tag="ot", name="ot")
            for n in range(2):
                po_ps = psum_proj.tile([128, 512], F32, tag="proj", name="po_ps")
                for d in range(NMT):
                    nc.tensor.matmul(
                        po_ps[:],
                        ctxT[d][:, 128 * qt:128 * (qt + 1)],
                        wo_sb[d][:, 512 * n:512 * (n + 1)],
                        start=(d == 0), stop=(d == NMT - 1))
                nc.scalar.activation(ot[:, 512 * n:512 * (n + 1)], po_ps[:],
                                     mybir.ActivationFunctionType.Copy)
            nc.sync.dma_start(out_ap[128 * qt:128 * (qt + 1), :], ot[:])

        # ---- chunk 0 projections upfront (diagonal starts at kt=0) ----
        for m in range(NMT):
            proj_chunk(xq_c0, wq_sb, m, qT, 0, m)
        for m in range(NMT):
            proj_chunk(xk_c0, wk_sb, NMT + m, kT, 0, m)
        for sti in range(4):
            v_chunk(xv_c0, 0, sti)

        # x prefetch state: chunk qc's x tiles are loaded during chunk qc-1
        xk_next = xv_next = None

        # ---- attention per q-chunk ----
        for qc in range(NQC):
            xk_cur, xv_cur = xk_next, xv_next
            nkt = 4 * (qc + 1)
            ndiag = 4 * qc
            G = (HL // 2) * nkt  # global iteration count for this chunk

            # filler list: (deadline_g, closure); sorted by deadline.
            # Deadlines are emission-order iterations g = hp*nkt + kt by which
            # the closure must have been emitted (data-dependency order).
            items = []
            BIG = 10 ** 6
            if qc > 0:
                # this chunk's v tiles (st >= 1): ctx(kt=ndiag+s) emitted at
                # python iteration ndiag+s+4 (pend depth); keep margin 2
                for s in range(1, 4):
                    items.append((min(ndiag + s + 2, nkt),
                                  lambda s=s, x=xv_cur: v_chunk(x, qc, s)))
                # this chunk's kT m-groups (m >= 1): scores(kt=ndiag) of pair
                # m emitted at g = m*nkt + ndiag; margin 2
                for m in range(1, NMT):
                    items.append((m * nkt + ndiag - 2,
                                  lambda m=m, x=xk_cur: proj_chunk(x, wk_sb, NMT + m, kT, qc, m)))
            if qc + 1 < NQC:
                xq_c = load_xt_chunk(xq_ap, qc + 1, "q")
                xk_next = load_xt_chunk(xk_ap, qc + 1, "k")
                xv_next = load_xt_chunk(xv_ap, qc + 1, "v")
                for m in range(NMT):
                    items.append((BIG, lambda m=m, x=xq_c, q=qc + 1: proj_chunk(x, wq_sb, m, qT, q, m)))
                # hoist next chunk's first kT / v group into this chunk
                items.append((BIG, lambda x=xk_next, q=qc + 1: proj_chunk(x, wk_sb, NMT + 0, kT, q, 0)))
                items.append((BIG, lambda x=xv_next, q=qc + 1: v_chunk(x, q, 0)))
            if qc > 0:
                items += [(BIG, lambda qt=4 * (qc - 1) + j: emit_outproj(qt))
                          for j in range(4)]
            items.sort(key=lambda it: it[0])
            nit = len(items)
            ndone = 0

            for hp in range(HL // 2):
                heads = (2 * hp, 2 * hp + 1)
                ctx_ps = {h: psum_ctx.tile([128, 512], F32, tag="ctx",
                                           name=f"ctx{h}") for h in heads}

                def emit_scores_exp(kt):
                    qs = max(0, 128 * kt - 512 * qc)
                    sc_ps = psum_big.tile([128, 1024], F32, tag="big", name="sc")
                    for i in range(2):
                        po = 64 * i
                        nc.tensor.matmul(
                            sc_ps[:, 512 * i + qs:512 * (i + 1)],
                            kT[hp][po:po + HD, 128 * kt:128 * (kt + 1)],
                            qT[hp][po:po + HD, 512 * qc + qs:512 * (qc + 1)],
                            start=True, stop=True)
                    # late chunks are Scalar(exp)-bound: offload a slice of
                    # non-diagonal tiles to DVE via the exp bit-trick
                    offload = (qc == 2 and kt % 4 == 3 and kt < ndiag) or \
                              (qc == 3 and kt % 3 == 2 and kt < ndiag)
                    if offload:
                        it = sch_pool.tile([128, 1024], mybir.dt.int16,
                                           tag="it", name="it")
                        nc.vector.tensor_scalar(
                            it[:], sc_ps[:], SK16, SB16,
                            mybir.AluOpType.mult, mybir.AluOpType.add)
                        return it[:].bitcast(BF16)
                    et = exp_pool.tile([128, 1024], BF16, tag="et", name="et")
                    nc.scalar.activation(et[:, qs:1024], sc_ps[:, qs:1024],
                                         EXP, scale=SCALE)
                    return et

                def emit_ctx(kt, et):
                    qs = max(0, 128 * kt - 512 * qc)
                    diag = ndiag <= kt
                    for i, h in enumerate(heads):
                        if diag:  # mask k>q in the diagonal 128x128 block
                            nc.gpsimd.affine_select(
                                out=et[:, 512 * i + qs:512 * i + qs + 128],
                                in_=et[:, 512 * i + qs:512 * i + qs + 128],
                                compare_op=mybir.AluOpType.is_ge, fill=0.0,
                                base=0, pattern=[[1, 128]], channel_multiplier=-1)
                        nc.tensor.matmul(
                            ctx_ps[h][:, qs:512],
                            v_aug[kt][:, h, :],
                            et[:, 512 * i + qs:512 * (i + 1)],
                            start=(kt == 0), stop=(kt == nkt - 1))

                # software pipeline: scores/exp run ahead of ctx
                pend = []
                for kt in range(nkt):
                    pend.append((kt, emit_scores_exp(kt)))
                    if len(pend) > 4:
                        emit_ctx(*pend.pop(0))
                    # fillers: emit when due (deadline) or by uniform pacing
                    g = hp * nkt + kt + 1
                    want = (nit * g) // G
                    while ndone < nit and (ndone < want or items[ndone][0] <= g):
                        items[ndone][1]()
                        ndone += 1
                # force items due within this pair before draining ctx
                lim = (hp + 1) * nkt
                while ndone < nit and items[ndone][0] <= lim:
                    items[ndone][1]()
                    ndone += 1
                for pn in pend:
                    emit_ctx(*pn)

                # normalization: replicated sums (psum rows 64-127) -> SBUF
                # base 0 (the custom recip op requires base-0 operands),
                # reciprocal, then multiply straight out of PSUM
                for h in heads:
                    po = 64 * (h % 2)
                    sums = norm_pool.tile([HD, 512], F32, tag="sums", name="sums")
                    nc.vector.tensor_copy(sums[:], ctx_ps[h][64:128, :])
                    recip = norm_pool.tile([HD, 512], F32, tag="recip", name="recip")
                    nc.vector.reciprocal_approx_fast(recip[:], sums[:])
                    nc.vector.tensor_mul(
                        ctxT[hp][po:po + HD, 512 * qc:512 * (qc + 1)],
                        ctx_ps[h][0:HD, :], recip[:])

            while ndone < nit:
                items[ndone][1]()
                ndone += 1

        for qt in range(4 * (NQC - 1), 4 * NQC):
            emit_outproj(qt)

    nc.compile()
    return nc


def _shard(inputs):
    in_maps = []
    for c in range(N_CORES):
        b, g = c // 2, c % 2
        sl = slice(512 * g, 512 * (g + 1))
        wqkv = np.concatenate([
            inputs["Wq"][:, sl], inputs["Wk"][:, sl], inputs["Wv"][:, sl]],
            axis=0).astype(ml_dtypes.bfloat16)
        bqk = np.empty((128, 8), np.float32)
        for m in range(4):
            bqk[:, m] = inputs["bq"][sl][128 * m:128 * (m + 1)]
            bqk[:, 4 + m] = inputs["bk"][sl][128 * m:128 * (m + 1)]
        in_maps.append({
            "xqt": np.ascontiguousarray(inputs["inputs_q"][b].T.astype(ml_dtypes.bfloat16)),
            "xkt": np.ascontiguousarray(inputs["inputs_k"][b].T.astype(ml_dtypes.bfloat16)),
            "xvt": np.ascontiguousarray(inputs["inputs_v"][b].T.astype(ml_dtypes.bfloat16)),
            "wqkv": np.ascontiguousarray(wqkv),
            "bqk": bqk,
            "bvb": np.ascontiguousarray(
                np.broadcast_to(inputs["bv"][sl], (128, 512))).astype(np.float32),
            "wo": np.ascontiguousarray(inputs["Wo"][sl, :].astype(ml_dtypes.bfloat16)),
        })
    return in_maps


def kernel(**inputs):
    global _compiled
    inputs = {k: np.asarray(v, dtype=np.float32) for k, v in inputs.items()}
    if _compiled is None:
        _compiled = _build()
    nc = _compiled
    in_maps = _shard(inputs)
    res = run_bass_kernel_spmd(nc, in_maps, list(range(N_CORES)),
                               trace=bool(int(__import__("os").environ.get("BASS_TRACE", "0"))))
    kernel.last_results = res
    B = 4
    out = np.empty((B, S, D), np.float32)
    for b in range(B):
        out[b] = (res.results[2 * b]["out"].astype(np.float32)
                  + res.results[2 * b + 1]["out"].astype(np.float32))
    out += inputs["bo"][None, None, :]
    return out
